# revision 10
# baseline (speedup 1.0000x reference)
"""Distributed GATv2 (2-layer + output proj) Bass kernel for 8 TRN2 NeuronCores.

Strategy (dst-node sharding, per the standard GNN graph-parallel recipe):
  - Nodes are partitioned across 8 cores (1250 each, padded to 1280).
  - Each core computes the src/dst/residual projections for its own nodes,
    then an AllGather replicates the src-side projected features so every
    core can gather arbitrary src rows for its incoming edges.
  - Edges live on the core that owns their dst node, grouped into dst blocks
    of 128; edge softmax + weighted aggregation for a block accumulate in
    PSUM via one-hot selection matmuls (segment-sum on the PE array).
  - The GATv2 score e = a . leaky_relu(fs[u]+fd[v]) is computed with the
    identity  leaky_relu(x) = 0.6x + 0.4|x|  (slope 0.2)  after folding
    diag(a) into the projection weights:
        e = 0.6*(p_u + q_v) + 0.4*(sum_pos |u'| - sum_neg |u'|)
    where u' = a.(fs+fd) comes straight from the gathers, the +/- split is a
    host-side column permutation, per-node sums p,q are extra matmul columns,
    and the |.| row-sums come free from activation accum_out.
  - softmax uses exp(e) directly (no max subtraction; |e| is O(1) here, and
    softmax is shift-invariant so this matches the reference mathematically).
"""

import numpy as np
import ml_dtypes

N_NODES = 10000
N_EDGES = 80000
H = 4
IN, D1, D2, OUT = 128, 512, 32, 64
C = 8                      # cores
NL = N_NODES // C          # 1250 nodes per core
BLK = 128
NBLK = -(-NL // BLK)       # 10 dst blocks per core
NLP = NBLK * BLK           # 1280 padded nodes per core
F1 = H * D1                # 2048
F2 = H * D2                # 128
W1ROW = F1 + 4             # gathered row width layer 1 (features + p/q)
W2ROW = F2 + 4             # layer 2

BF = ml_dtypes.bfloat16

_cache = {}


# ----------------------------------------------------------------- host prep
def _fold(attn, Ws, bs, Wd, bd):
    """Fold diag(a) into W/b, permute columns pos-first per head."""
    D = attn.shape[1]
    a = attn.reshape(H * D)
    perm = np.concatenate(
        [np.argsort(attn[h] < 0, kind="stable") + h * D for h in range(H)]
    )
    npos = [int((attn[h] >= 0).sum()) for h in range(H)]
    Wsp = (Ws * a[None, :])[:, perm]
    bsp = (bs * a)[perm]
    Wdp = (Wd * a[None, :])[:, perm]
    bdp = (bd * a)[perm]
    # per-head column sums give p/q as extra matmul outputs
    ws_p = np.stack([Wsp[:, h * D:(h + 1) * D].sum(1) for h in range(H)], 1)
    bs_p = np.array([bsp[h * D:(h + 1) * D].sum() for h in range(H)], np.float32)
    wd_q = np.stack([Wdp[:, h * D:(h + 1) * D].sum(1) for h in range(H)], 1)
    bd_q = np.array([bdp[h * D:(h + 1) * D].sum() for h in range(H)], np.float32)
    inva = (1.0 / a[perm]).astype(np.float32)
    return perm, npos, Wsp, bsp, Wdp, bdp, ws_p, bs_p, wd_q, bd_q, inva


def _edge_arrays(src, dst):
    """Per-core edge tiles: block-grouped, padded. Returns (T_BLK, per-core dict)."""
    order = np.argsort(dst, kind="stable")
    src_s, dst_s = src[order], dst[order]
    cores = []
    for c in range(C):
        m = (dst_s >= c * NL) & (dst_s < (c + 1) * NL)
        cores.append((src_s[m], dst_s[m] - c * NL))
    t_blk = 1
    counts = []
    for s_c, dl_c in cores:
        cnt = [int(((dl_c >= b * BLK) & (dl_c < (b + 1) * BLK)).sum())
               for b in range(NBLK)]
        counts.append(cnt)
        t_blk = max(t_blk, max(-(-n // 128) for n in cnt) if cnt else 1)
    per_core = []
    for c in range(C):
        s_c, dl_c = cores[c]
        srcpos = np.zeros((NBLK, t_blk * 128), np.int32)
        fdrow = np.zeros((NBLK, t_blk * 128), np.int32)
        dloc = np.full((NBLK, t_blk * 128), 200.0, np.float32)
        for b in range(NBLK):
            m = (dl_c >= b * BLK) & (dl_c < (b + 1) * BLK)
            sb, db = s_c[m], dl_c[m]
            n = len(sb)
            srcpos[b, :n] = (sb // NL) * NLP + (sb % NL)
            fdrow[b, :n] = db
            dloc[b, :n] = (db - b * BLK).astype(np.float32)
        per_core.append(dict(
            srcpos=srcpos.reshape(-1, 1),
            fdrow=fdrow.reshape(-1, 1),
            dstloc=dloc.reshape(-1, 1),
        ))
    return t_blk, per_core


# ------------------------------------------------------------------- builder
def _build(meta):
    import concourse.bass as bass
    import concourse.mybir as mybir
    import concourse.tile as tile
    from concourse import bacc
    from concourse.masks import make_identity

    dt = mybir.dt
    AF = mybir.ActivationFunctionType
    ALU = mybir.AluOpType
    T_BLK = meta["t_blk"]
    npos1, npos2 = meta["npos1"], meta["npos2"]
    NT = NBLK * T_BLK
    W1CAT = 2 * W1ROW + F1      # 6152
    W2CAT = 2 * W2ROW + F2      # 392

    nc = bacc.Bacc("TRN2", target_bir_lowering=False, debug=False, num_devices=C)

    def din(name, shape, dtype):
        return nc.dram_tensor(name, shape, dtype, kind="ExternalInput").ap()

    h0T = din("h0T", [IN, NLP], dt.float32)
    W1cat = din("W1cat", [IN, W1CAT], dt.float32)
    b1cat = din("b1cat", [1, W1CAT], dt.float32)
    W2cat = din("W2cat", [F1, W2CAT], dt.bfloat16)
    b2cat = din("b2cat", [1, W2CAT], dt.bfloat16)
    Wout = din("Wout", [F2, OUT], dt.bfloat16)
    bout = din("bout", [1, OUT], dt.bfloat16)
    inva1 = din("inva1", [1, F1], dt.float32)
    inva2 = din("inva2", [1, F2], dt.float32)
    srcpos = din("srcpos", [NT * 128, 1], dt.int32)
    fdrow = din("fdrow", [NT * 128, 1], dt.int32)
    dstloc = din("dstloc", [NT * 128, 1], dt.float32)
    out_d = nc.dram_tensor("out", [NLP, OUT], dt.float32, kind="ExternalOutput").ap()

    groups = [list(range(C))]

    from contextlib import ExitStack
    with tile.TileContext(nc) as tc, ExitStack() as stack:
        cst = stack.enter_context(tc.tile_pool(name="cst", bufs=1))
        dram = stack.enter_context(tc.tile_pool(name="dram", bufs=1, space="DRAM"))

        # ---------------- constants
        iota_i = cst.tile([128, 128], dt.int32)
        nc.gpsimd.iota(iota_i[:], pattern=[[1, 128]], base=0, channel_multiplier=0)
        iota_f = cst.tile([128, 128], dt.float32)
        nc.vector.tensor_copy(iota_f[:], iota_i[:])
        ident_bf = cst.tile([128, 128], dt.bfloat16)
        make_identity(nc, ident_bf[:])
        ones1f = cst.tile([1, 128], dt.float32)
        nc.any.memset(ones1f[:], 1.0)
        ones1b = cst.tile([1, 128], dt.bfloat16)
        nc.any.memset(ones1b[:], 1.0)
        onesPb = cst.tile([128, 1], dt.bfloat16)
        nc.any.memset(onesPb[:], 1.0)
        h0T_sb = cst.tile([IN, NLP], dt.float32)
        nc.sync.dma_start(out=h0T_sb[:], in_=h0T)
        W1cat_sb = cst.tile([IN, W1CAT], dt.float32)
        nc.sync.dma_start(out=W1cat_sb[:], in_=W1cat)
        b1cat_sb = cst.tile([1, W1CAT], dt.float32)
        nc.sync.dma_start(out=b1cat_sb[:], in_=b1cat)
        b2cat_sb = cst.tile([1, W2CAT], dt.bfloat16)
        nc.sync.dma_start(out=b2cat_sb[:], in_=b2cat)
        Wout_sb = cst.tile([F2, OUT], dt.bfloat16)
        nc.sync.dma_start(out=Wout_sb[:], in_=Wout)
        bout_sb = cst.tile([1, OUT], dt.bfloat16)
        nc.sync.dma_start(out=bout_sb[:], in_=bout)
        inva1_sb = cst.tile([128, F1], dt.float32)
        nc.sync.dma_start(out=inva1_sb[:], in_=inva1[0:1, :].to_broadcast([128, F1]))
        inva2_sb = cst.tile([128, F2], dt.float32)
        nc.sync.dma_start(out=inva2_sb[:], in_=inva2[0:1, :].to_broadcast([128, F2]))
        W2_sb = []
        for j in range(F1 // 128):
            t = cst.tile([128, W2CAT], dt.bfloat16, name=f"W2sb{j}")
            nc.sync.dma_start(out=t[:], in_=W2cat[j * 128:(j + 1) * 128, :])
            W2_sb.append(t)

        # ---------------- DRAM intermediates
        cc1_in = dram.tile([NLP, W1ROW], dt.bfloat16)
        shared_as = "Shared" if C >= 8 else "Local"
        fs1_full = dram.tile([C * NLP, W1ROW], dt.bfloat16, addr_space=shared_as)
        fd1q = dram.tile([NLP, W1ROW], dt.bfloat16)
        res1 = dram.tile([NLP, F1], dt.float32)
        h1T = dram.tile([F1, NLP], dt.bfloat16)
        cc2_in = dram.tile([NLP, W2ROW], dt.bfloat16)
        fs2_full = dram.tile([C * NLP, W2ROW], dt.bfloat16, addr_space=shared_as)
        fd2q = dram.tile([NLP, W2ROW], dt.bfloat16)
        res2 = dram.tile([NLP, F2], dt.float32)

        # ============ PHASE A: layer-1 projections ============
        # column regions of the [128, W1CAT] result:
        #   [0, W1ROW)              -> cc1_in   (fs' + p)        bf16
        #   [W1ROW, 2*W1ROW)        -> fd1q     (fd' + q)        bf16
        #   [2*W1ROW, W1CAT)        -> res1                       f32
        with tc.tile_pool(name="psA", bufs=4, space="PSUM") as psA, \
             tc.tile_pool(name="stA", bufs=2) as stAp, \
             tc.tile_pool(name="stR", bufs=2) as stRp:
            CH = 512
            nch = -(-W1CAT // CH)
            for nt in range(NBLK):
                stA = stAp.tile([128, 2 * W1ROW], dt.bfloat16, name="stA")
                stR = stRp.tile([128, F1], dt.float32, name="stR")
                for j in range(nch):
                    c0 = j * CH
                    w = min(CH, W1CAT - c0)
                    pa = psA.tile([128, CH], dt.float32, name="pa")
                    nc.tensor.matmul(pa[:, :w], lhsT=ones1f[:],
                                     rhs=b1cat_sb[:, c0:c0 + w],
                                     start=True, stop=False)
                    nc.tensor.matmul(pa[:, :w],
                                     lhsT=h0T_sb[:, nt * 128:(nt + 1) * 128],
                                     rhs=W1cat_sb[:, c0:c0 + w],
                                     start=False, stop=True)
                    # evict by region intersection
                    for r0, r1, dst_t, doff in (
                        (0, 2 * W1ROW, stA, 0),
                        (2 * W1ROW, W1CAT, stR, 2 * W1ROW),
                    ):
                        lo, hi = max(c0, r0), min(c0 + w, r1)
                        if lo < hi:
                            nc.scalar.activation(
                                dst_t[:, lo - doff:hi - doff],
                                pa[:, lo - c0:hi - c0], AF.Copy)
                rows = slice(nt * 128, (nt + 1) * 128)
                nc.sync.dma_start(out=cc1_in[rows, :], in_=stA[:, 0:W1ROW])
                nc.sync.dma_start(out=fd1q[rows, :], in_=stA[:, W1ROW:2 * W1ROW])
                nc.sync.dma_start(out=res1[rows, :], in_=stR[:])

        # ============ PHASE B: AllGather layer-1 src features ============
        nc.gpsimd.collective_compute(
            "AllGather", mybir.AluOpType.bypass, replica_groups=groups,
            ins=[cc1_in[:]], outs=[fs1_full[:]])

        # ============ edge-phase helper (shared by both layers) ============
        def edge_phase(FSfull, FDtab, RES, D, wrow, npos, inva_sb, nfeat,
                       epilogue):
            """Per dst-block softmax + aggregation.  epilogue(b, h_blk_sbuf_f32)."""
            with tc.tile_pool(name="eFS", bufs=3) as pFS, \
                 tc.tile_pool(name="eFD", bufs=3) as pFD, \
                 tc.tile_pool(name="eU", bufs=2) as pU, \
                 tc.tile_pool(name="eSC", bufs=2) as pSC, \
                 tc.tile_pool(name="eSm", bufs=3) as pSm, \
                 tc.tile_pool(name="eW", bufs=2) as pW, \
                 tc.tile_pool(name="psN", bufs=1, space="PSUM") as psN, \
                 tc.tile_pool(name="psS", bufs=1, space="PSUM") as psS, \
                 tc.tile_pool(name="eZ", bufs=2) as pZ, \
                 tc.tile_pool(name="psT", bufs=2, space="PSUM") as psT:
                for b in range(NBLK):
                    nps = [psN.tile([128, D], dt.float32, name=f"num{h}")
                           for h in range(H)]
                    sps = psS.tile([128, H], dt.float32, name="sps")
                    for t in range(T_BLK):
                        r0 = (b * T_BLK + t) * 128
                        idx_t = pSm.tile([128, 1], dt.int32, name="idx")
                        nc.sync.dma_start(out=idx_t[:], in_=srcpos[r0:r0 + 128, :])
                        fdr_t = pSm.tile([128, 1], dt.int32, name="fdr")
                        nc.sync.dma_start(out=fdr_t[:], in_=fdrow[r0:r0 + 128, :])
                        dl_t = pSm.tile([128, 1], dt.float32, name="dl")
                        nc.sync.dma_start(out=dl_t[:], in_=dstloc[r0:r0 + 128, :])
                        FS = pFS.tile([128, wrow], dt.bfloat16, name="FS")
                        nc.gpsimd.indirect_dma_start(
                            out=FS[:], out_offset=None, in_=FSfull[:],
                            in_offset=bass.IndirectOffsetOnAxis(ap=idx_t[:, :1], axis=0))
                        FD = pFD.tile([128, wrow], dt.bfloat16, name="FD")
                        nc.gpsimd.indirect_dma_start(
                            out=FD[:], out_offset=None, in_=FDtab[:],
                            in_offset=bass.IndirectOffsetOnAxis(ap=fdr_t[:, :1], axis=0))
                        u = pU.tile([128, wrow], dt.bfloat16, name="u")
                        nc.vector.tensor_tensor(out=u[:], in0=FS[:], in1=FD[:],
                                                op=ALU.add)
                        scr = pSC.tile([128, nfeat], dt.bfloat16, name="scr")
                        acc = pSm.tile([128, 2 * H], dt.float32, name="acc")
                        for h in range(H):
                            sl = [(h * D, h * D + npos[h], h),
                                  (h * D + npos[h], (h + 1) * D, H + h)]
                            for (a0, a1, col) in sl:
                                if a0 == a1:
                                    nc.vector.memset(acc[:, col:col + 1], 0.0)
                                else:
                                    nc.scalar.activation(
                                        scr[:, a0:a1], u[:, a0:a1], AF.Abs,
                                        accum_out=acc[:, col:col + 1])
                        d4 = pSm.tile([128, H], dt.float32, name="d4")
                        nc.vector.tensor_tensor(out=d4[:], in0=acc[:, 0:H],
                                                in1=acc[:, H:2 * H], op=ALU.subtract)
                        pq = pSm.tile([128, H], dt.float32, name="pq")
                        nc.vector.tensor_scalar(out=pq[:], in0=u[:, nfeat:nfeat + H],
                                                scalar1=1.5, scalar2=None,
                                                op0=ALU.mult)
                        t2 = pSm.tile([128, H], dt.float32, name="t2")
                        nc.vector.tensor_tensor(out=t2[:], in0=pq[:], in1=d4[:],
                                                op=ALU.add)
                        wv = pSm.tile([128, H], dt.float32, name="wv")
                        nc.scalar.activation(wv[:], t2[:], AF.Exp, scale=0.4)
                        wvb = pSm.tile([128, H], dt.bfloat16, name="wvb")
                        nc.vector.tensor_copy(wvb[:], wv[:])
                        mask = pW.tile([128, 128], dt.bfloat16, name="mask")
                        nc.vector.tensor_scalar(
                            out=mask[:], in0=iota_f[:],
                            scalar1=dl_t[:, 0:1], scalar2=None,
                            op0=ALU.is_equal)
                        nc.tensor.matmul(sps[:], lhsT=mask[:], rhs=wvb[:],
                                         start=(t == 0), stop=(t == T_BLK - 1))
                        for h in range(H):
                            wsel = pW.tile([128, 128], dt.bfloat16, name=f"wsel{h}")
                            nc.vector.tensor_scalar(
                                out=wsel[:], in0=iota_f[:],
                                scalar1=dl_t[:, 0:1], scalar2=wv[:, h:h + 1],
                                op0=ALU.is_equal, op1=ALU.mult)
                            nc.tensor.matmul(nps[h][:], lhsT=wsel[:],
                                             rhs=FS[:, h * D:(h + 1) * D],
                                             start=(t == 0), stop=(t == T_BLK - 1))
                    # ---- block epilogue: h_blk = relu(num/s * inva + res)
                    seps = pSm.tile([128, H], dt.float32, name="seps")
                    nc.vector.tensor_scalar(out=seps[:], in0=sps[:], scalar1=1e-20,
                                            scalar2=None, op0=ALU.add)
                    rcp = pSm.tile([128, H], dt.float32, name="rcp")
                    nc.vector.reciprocal(rcp[:], seps[:])
                    z = pZ.tile([128, nfeat], dt.float32, name="z")
                    for h in range(H):
                        nc.scalar.activation(z[:, h * D:(h + 1) * D], nps[h][:],
                                             AF.Copy, scale=rcp[:, h:h + 1])
                    zi = pZ.tile([128, nfeat], dt.float32, name="zi")
                    nc.vector.tensor_tensor(out=zi[:], in0=z[:], in1=inva_sb[:],
                                            op=ALU.mult)
                    res_b = pZ.tile([128, nfeat], dt.float32, name="resb")
                    nc.sync.dma_start(out=res_b[:],
                                      in_=RES[b * 128:(b + 1) * 128, :])
                    zr = pZ.tile([128, nfeat], dt.float32, name="zr")
                    nc.vector.tensor_tensor(out=zr[:], in0=zi[:], in1=res_b[:],
                                            op=ALU.add)
                    hb = pZ.tile([128, nfeat], dt.bfloat16, name="hb")
                    nc.scalar.activation(hb[:], zr[:], AF.Relu)
                    epilogue(b, hb, psT, pZ)

        # ============ PHASE C: layer-1 edges ============
        def epi1(b, hb, psT, pZ):
            # transpose [128, F1] -> h1T dram
            for j in range(F1 // 128):
                pt = psT.tile([128, 128], dt.bfloat16, name="pt")
                nc.tensor.transpose(pt[:], hb[:, j * 128:(j + 1) * 128], ident_bf[:])
                ht = pZ.tile([128, 128], dt.bfloat16, name="ht")
                nc.scalar.activation(ht[:], pt[:], AF.Copy)
                nc.sync.dma_start(
                    out=h1T[j * 128:(j + 1) * 128, b * 128:(b + 1) * 128],
                    in_=ht[:])

        edge_phase(fs1_full, fd1q, res1, D1, W1ROW, npos1, inva1_sb, F1, epi1)

        # ============ PHASE D: layer-2 projections ============
        with tc.tile_pool(name="psD", bufs=2, space="PSUM") as psD, \
             tc.tile_pool(name="stD", bufs=2) as stDp, \
             tc.tile_pool(name="lhD", bufs=3) as lhDp:
            for nt in range(NBLK):
                pd = psD.tile([128, W2CAT], dt.float32, name="pd")
                nc.tensor.matmul(pd[:], lhsT=ones1b[:], rhs=b2cat_sb[:],
                                 start=True, stop=False)
                for j in range(F1 // 128):
                    lh = lhDp.tile([128, 128], dt.bfloat16, name="lh")
                    nc.sync.dma_start(
                        out=lh[:],
                        in_=h1T[j * 128:(j + 1) * 128, nt * 128:(nt + 1) * 128])
                    nc.tensor.matmul(pd[:], lhsT=lh[:], rhs=W2_sb[j][:],
                                     start=False, stop=(j == F1 // 128 - 1))
                stD = stDp.tile([128, 2 * W2ROW], dt.bfloat16, name="stD")
                nc.scalar.activation(stD[:], pd[:, 0:2 * W2ROW], AF.Copy)
                stR2 = stDp.tile([128, F2], dt.float32, name="stR2")
                nc.scalar.activation(stR2[:], pd[:, 2 * W2ROW:W2CAT], AF.Copy)
                rows = slice(nt * 128, (nt + 1) * 128)
                nc.sync.dma_start(out=cc2_in[rows, :], in_=stD[:, 0:W2ROW])
                nc.sync.dma_start(out=fd2q[rows, :], in_=stD[:, W2ROW:2 * W2ROW])
                nc.sync.dma_start(out=res2[rows, :], in_=stR2[:])

        # ============ PHASE E: AllGather layer-2 ============
        nc.gpsimd.collective_compute(
            "AllGather", mybir.AluOpType.bypass, replica_groups=groups,
            ins=[cc2_in[:]], outs=[fs2_full[:]])

        # ============ PHASE F: layer-2 edges + output proj ============
        with tc.tile_pool(name="psO", bufs=1, space="PSUM") as psO, \
             tc.tile_pool(name="oSB", bufs=2) as oSB:
            def epi2(b, hb, psT, pZ):
                pt = psT.tile([128, 128], dt.bfloat16, name="pt2")
                nc.tensor.transpose(pt[:], hb[:], ident_bf[:])
                ht = pZ.tile([128, 128], dt.bfloat16, name="ht2")
                nc.scalar.activation(ht[:], pt[:], AF.Copy)
                po = psO.tile([128, OUT], dt.float32, name="po")
                nc.tensor.matmul(po[:], lhsT=ones1b[:], rhs=bout_sb[:],
                                 start=True, stop=False)
                nc.tensor.matmul(po[:], lhsT=ht[:], rhs=Wout_sb[:],
                                 start=False, stop=True)
                ob = oSB.tile([128, OUT], dt.float32, name="ob")
                nc.scalar.activation(ob[:], po[:], AF.Copy)
                nc.sync.dma_start(out=out_d[b * 128:(b + 1) * 128, :], in_=ob[:])

            edge_phase(fs2_full, fd2q, res2, D2, W2ROW, npos2, inva2_sb, F2, epi2)

    nc.compile()
    return nc


# -------------------------------------------------------------------- kernel
def _prepare(inputs):
    inputs = {k: np.asarray(v) for k, v in inputs.items()}
    f32 = np.float32

    feat = inputs["features"].astype(f32)
    src = inputs["src"].astype(np.int64)
    dst = inputs["dst"].astype(np.int64)

    (perm1, npos1, W1sp, b1sp, W1dp, b1dp, w1p, b1p, w1q, b1q, inva1) = _fold(
        np.asarray(inputs["attn1"], f32), inputs["W1s"].astype(f32),
        inputs["b1s"].astype(f32), inputs["W1d"].astype(f32),
        inputs["b1d"].astype(f32))
    W1r_p = inputs["W1r"].astype(f32)[:, perm1]
    b1r_p = inputs["b1r"].astype(f32)[perm1]

    (perm2, npos2, W2sp, b2sp, W2dp, b2dp, w2p, b2p, w2q, b2q, inva2) = _fold(
        np.asarray(inputs["attn2"], f32), inputs["W2s"].astype(f32)[perm1],
        inputs["b2s"].astype(f32), inputs["W2d"].astype(f32)[perm1],
        inputs["b2d"].astype(f32))
    W2r_p = inputs["W2r"].astype(f32)[perm1][:, perm2]
    b2r_p = inputs["b2r"].astype(f32)[perm2]
    Wout_p = inputs["Wout"].astype(f32)[perm2]
    bout_v = inputs["bout"].astype(f32)

    W1cat = np.concatenate([W1sp, w1p, W1dp, w1q, W1r_p], 1)          # [128, 6152]
    b1cat = np.concatenate([b1sp, b1p, b1dp, b1q, b1r_p])[None, :]
    W2cat = np.concatenate([W2sp, w2p, W2dp, w2q, W2r_p], 1)          # [2048, 392]
    b2cat = np.concatenate([b2sp, b2p, b2dp, b2q, b2r_p])[None, :]

    t_blk, edge_pc = _edge_arrays(src, dst)

    key = (t_blk, tuple(npos1), tuple(npos2))
    if key not in _cache:
        _cache[key] = _build(dict(t_blk=t_blk, npos1=npos1, npos2=npos2))
    nc = _cache[key]

    in_maps = []
    for c in range(C):
        h0 = np.zeros((NLP, IN), f32)
        h0[:NL] = feat[c * NL:(c + 1) * NL]
        m = dict(
            h0T=np.ascontiguousarray(h0.T),
            W1cat=W1cat, b1cat=b1cat,
            W2cat=W2cat.astype(BF), b2cat=b2cat.astype(BF),
            Wout=Wout_p.astype(BF), bout=bout_v[None, :].astype(BF),
            inva1=inva1[None, :], inva2=inva2[None, :],
            **edge_pc[c],
        )
        in_maps.append(m)
    return nc, in_maps


def kernel(**inputs):
    nc, in_maps = _prepare(inputs)
    from concourse.bass_utils import run_bass_kernel_spmd
    res = run_bass_kernel_spmd(nc, in_maps, core_ids=list(range(C)))
    out = np.concatenate([res.results[c]["out"][:NL] for c in range(C)], 0)
    return out.astype(np.float32)


# revision 14
# speedup vs baseline: 1.3855x; 1.3855x over previous
"""Distributed GATv2 (2-layer + output proj) Bass kernel for 8 TRN2 NeuronCores.

Strategy (dst-node sharding, per the standard GNN graph-parallel recipe):
  - Nodes are partitioned across 8 cores (1250 each, padded to 1280).
  - Each core computes the src/dst/residual projections for its own nodes,
    then an AllGather replicates the src-side projected features so every
    core can gather arbitrary src rows for its incoming edges.
  - Edges live on the core that owns their dst node, grouped into dst blocks
    of 128; edge softmax + weighted aggregation for a block accumulate in
    PSUM via one-hot selection matmuls (segment-sum on the PE array).
  - The GATv2 score e = a . leaky_relu(fs[u]+fd[v]) is computed with the
    identity  leaky_relu(x) = 0.6x + 0.4|x|  (slope 0.2)  after folding
    diag(a) into the projection weights:
        e = 0.6*(p_u + q_v) + 0.4*(sum_pos |u'| - sum_neg |u'|)
    where u' = a.(fs+fd) comes straight from the gathers, the +/- split is a
    host-side column permutation, per-node sums p,q are extra matmul columns,
    and the |.| row-sums come free from activation accum_out.
  - softmax uses exp(e) directly (no max subtraction; |e| is O(1) here, and
    softmax is shift-invariant so this matches the reference mathematically).
"""

import numpy as np
import ml_dtypes

N_NODES = 10000
N_EDGES = 80000
H = 4
IN, D1, D2, OUT = 128, 512, 32, 64
C = 8                      # cores
NL = N_NODES // C          # 1250 nodes per core
BLK = 128
NBLK = -(-NL // BLK)       # 10 dst blocks per core
NLP = NBLK * BLK           # 1280 padded nodes per core
F1 = H * D1                # 2048
F2 = H * D2                # 128
W1ROW = F1 + 4             # gathered row width layer 1 (features + p/q)
W2ROW = F2 + 4             # layer 2

BF = ml_dtypes.bfloat16

_cache = {}


# ----------------------------------------------------------------- host prep
def _fold(attn, Ws, bs, Wd, bd):
    """Fold diag(a) into W/b, permute columns pos-first per head."""
    D = attn.shape[1]
    a = attn.reshape(H * D)
    perm = np.concatenate(
        [np.argsort(attn[h] < 0, kind="stable") + h * D for h in range(H)]
    )
    npos = [int((attn[h] >= 0).sum()) for h in range(H)]
    Wsp = (Ws * a[None, :])[:, perm]
    bsp = (bs * a)[perm]
    Wdp = (Wd * a[None, :])[:, perm]
    bdp = (bd * a)[perm]
    # per-head column sums give p/q as extra matmul outputs
    ws_p = np.stack([Wsp[:, h * D:(h + 1) * D].sum(1) for h in range(H)], 1)
    bs_p = np.array([bsp[h * D:(h + 1) * D].sum() for h in range(H)], np.float32)
    wd_q = np.stack([Wdp[:, h * D:(h + 1) * D].sum(1) for h in range(H)], 1)
    bd_q = np.array([bdp[h * D:(h + 1) * D].sum() for h in range(H)], np.float32)
    inva = (1.0 / a[perm]).astype(np.float32)
    return perm, npos, Wsp, bsp, Wdp, bdp, ws_p, bs_p, wd_q, bd_q, inva


def _edge_arrays(src, dst):
    """Per-core edge tiles: block-grouped, padded. Returns (T_BLK, per-core dict)."""
    order = np.argsort(dst, kind="stable")
    src_s, dst_s = src[order], dst[order]
    cores = []
    for c in range(C):
        m = (dst_s >= c * NL) & (dst_s < (c + 1) * NL)
        cores.append((src_s[m], dst_s[m] - c * NL))
    t_blk = 1
    counts = []
    for s_c, dl_c in cores:
        cnt = [int(((dl_c >= b * BLK) & (dl_c < (b + 1) * BLK)).sum())
               for b in range(NBLK)]
        counts.append(cnt)
        t_blk = max(t_blk, max(-(-n // 128) for n in cnt) if cnt else 1)
    per_core = []
    for c in range(C):
        s_c, dl_c = cores[c]
        srcpos = np.zeros((NBLK, t_blk * 128), np.int32)
        fdrow = np.zeros((NBLK, t_blk * 128), np.int32)
        dloc = np.full((NBLK, t_blk * 128), 200.0, np.float32)
        for b in range(NBLK):
            m = (dl_c >= b * BLK) & (dl_c < (b + 1) * BLK)
            sb, db = s_c[m], dl_c[m]
            n = len(sb)
            srcpos[b, :n] = (sb // NL) * NLP + (sb % NL)
            fdrow[b, :n] = db
            dloc[b, :n] = (db - b * BLK).astype(np.float32)
        pk = np.zeros((NBLK * t_blk * 128, 4), np.int32)
        pk[:, 0] = srcpos.reshape(-1)
        pk[:, 1] = fdrow.reshape(-1)
        pk[:, 2] = dloc.reshape(-1).astype(np.int32)
        per_core.append(dict(idxpk=pk))
    return t_blk, per_core


# ------------------------------------------------------------------- builder
def _build(meta):
    import concourse.bass as bass
    import concourse.mybir as mybir
    import concourse.tile as tile
    from concourse import bacc
    from concourse.masks import make_identity

    dt = mybir.dt
    AF = mybir.ActivationFunctionType
    ALU = mybir.AluOpType
    T_BLK = meta["t_blk"]
    npos1, npos2 = meta["npos1"], meta["npos2"]
    NT = NBLK * T_BLK
    W1CAT = 2 * W1ROW + F1      # 6152
    W2CAT = 2 * W2ROW + F2      # 392

    nc = bacc.Bacc("TRN2", target_bir_lowering=False, debug=False, num_devices=C)

    def din(name, shape, dtype):
        return nc.dram_tensor(name, shape, dtype, kind="ExternalInput").ap()

    h0T = din("h0T", [IN, NLP], dt.float32)
    W1cat = din("W1cat", [IN, W1CAT], dt.float32)
    b1cat = din("b1cat", [1, W1CAT], dt.float32)
    W2cat = din("W2cat", [F1, W2CAT], dt.bfloat16)
    b2cat = din("b2cat", [1, W2CAT], dt.float32)
    Wout = din("Wout", [F2, OUT], dt.bfloat16)
    bout = din("bout", [1, OUT], dt.float32)
    inva1 = din("inva1", [1, F1], dt.float32)
    inva2 = din("inva2", [1, F2], dt.float32)
    idxpk = din("idxpk", [NT * 128, 4], dt.int32)
    out_d = nc.dram_tensor("out", [NLP, OUT], dt.float32, kind="ExternalOutput").ap()

    groups = [list(range(C))]

    from contextlib import ExitStack
    with tile.TileContext(nc) as tc, ExitStack() as stack:
        cst = stack.enter_context(tc.tile_pool(name="cst", bufs=1))
        dram = stack.enter_context(tc.tile_pool(name="dram", bufs=1, space="DRAM"))

        # ---------------- constants
        iota_i = cst.tile([128, 128], dt.int32)
        nc.gpsimd.iota(iota_i[:], pattern=[[1, 128]], base=0, channel_multiplier=0)
        iota_f = cst.tile([128, 128], dt.float32)
        nc.vector.tensor_copy(iota_f[:], iota_i[:])
        ident_bf = cst.tile([128, 128], dt.bfloat16)
        make_identity(nc, ident_bf[:])
        h0T_sb = cst.tile([IN, NLP], dt.float32)
        nc.sync.dma_start(out=h0T_sb[:], in_=h0T)
        W1cat_sb = cst.tile([IN, W1CAT], dt.float32)
        nc.sync.dma_start(out=W1cat_sb[:], in_=W1cat)
        b1B = cst.tile([128, W1CAT], dt.float32)
        nc.sync.dma_start(out=b1B[:], in_=b1cat[0:1, :].to_broadcast([128, W1CAT]))
        b2B = cst.tile([128, W2CAT], dt.float32)
        nc.sync.dma_start(out=b2B[:], in_=b2cat[0:1, :].to_broadcast([128, W2CAT]))
        Wout_sb = cst.tile([F2, OUT], dt.bfloat16)
        nc.sync.dma_start(out=Wout_sb[:], in_=Wout)
        boutB = cst.tile([128, OUT], dt.float32)
        nc.sync.dma_start(out=boutB[:], in_=bout[0:1, :].to_broadcast([128, OUT]))
        inva1_sb = cst.tile([128, F1], dt.float32)
        nc.sync.dma_start(out=inva1_sb[:], in_=inva1[0:1, :].to_broadcast([128, F1]))
        inva2_sb = cst.tile([128, F2], dt.float32)
        nc.sync.dma_start(out=inva2_sb[:], in_=inva2[0:1, :].to_broadcast([128, F2]))
        W2_sb = []
        for j in range(F1 // 128):
            t = cst.tile([128, W2CAT], dt.bfloat16, name=f"W2sb{j}")
            nc.sync.dma_start(out=t[:], in_=W2cat[j * 128:(j + 1) * 128, :])
            W2_sb.append(t)

        # ---------------- DRAM intermediates
        cc1_in = dram.tile([NLP, W1ROW], dt.bfloat16)
        shared_as = "Shared" if C >= 8 else "Local"
        fs1_full = dram.tile([C * NLP, W1ROW], dt.bfloat16, addr_space=shared_as)
        fd1q = dram.tile([NLP, W1ROW], dt.bfloat16)
        res1 = dram.tile([NLP, F1], dt.float32)
        h1T = dram.tile([F1, NLP], dt.bfloat16)
        cc2_in = dram.tile([NLP, W2ROW], dt.bfloat16)
        fs2_full = dram.tile([C * NLP, W2ROW], dt.bfloat16, addr_space=shared_as)
        fd2q = dram.tile([NLP, W2ROW], dt.bfloat16)
        res2 = dram.tile([NLP, F2], dt.float32)

        # ============ PHASE A: layer-1 projections ============
        # column regions of the [128, W1CAT] result:
        #   [0, W1ROW)              -> cc1_in   (fs' + p)        bf16
        #   [W1ROW, 2*W1ROW)        -> fd1q     (fd' + q)        bf16
        #   [2*W1ROW, W1CAT)        -> res1                       f32
        with tc.tile_pool(name="psA", bufs=4, space="PSUM") as psA, \
             tc.tile_pool(name="stA", bufs=2) as stAp, \
             tc.tile_pool(name="stR", bufs=2) as stRp:
            CH = 512
            nch = -(-W1CAT // CH)
            for nt in range(NBLK):
                stA = stAp.tile([128, 2 * W1ROW], dt.bfloat16, name="stA")
                stR = stRp.tile([128, F1], dt.float32, name="stR")
                for j in range(nch):
                    c0 = j * CH
                    w = min(CH, W1CAT - c0)
                    pa = psA.tile([128, CH], dt.float32, name="pa")
                    nc.tensor.matmul(pa[:, :w],
                                     lhsT=h0T_sb[:, nt * 128:(nt + 1) * 128],
                                     rhs=W1cat_sb[:, c0:c0 + w],
                                     start=True, stop=True)
                    # evict by region intersection, adding bias on the way out
                    for r0, r1, dst_t, doff in (
                        (0, 2 * W1ROW, stA, 0),
                        (2 * W1ROW, W1CAT, stR, 2 * W1ROW),
                    ):
                        lo, hi = max(c0, r0), min(c0 + w, r1)
                        if lo < hi:
                            nc.vector.tensor_tensor(
                                out=dst_t[:, lo - doff:hi - doff],
                                in0=pa[:, lo - c0:hi - c0],
                                in1=b1B[:, lo:hi], op=ALU.add)
                rows = slice(nt * 128, (nt + 1) * 128)
                nc.sync.dma_start(out=cc1_in[rows, :], in_=stA[:, 0:W1ROW])
                nc.sync.dma_start(out=fd1q[rows, :], in_=stA[:, W1ROW:2 * W1ROW])
                nc.sync.dma_start(out=res1[rows, :], in_=stR[:])

        # ============ PHASE B: AllGather layer-1 src features ============
        nc.gpsimd.collective_compute(
            "AllGather", mybir.AluOpType.bypass, replica_groups=groups,
            ins=[cc1_in[:]], outs=[fs1_full[:]])

        # ============ edge-phase helper (shared by both layers) ============
        def edge_phase(FSfull, FDtab, RES, D, wrow, npos, inva_sb, nfeat,
                       epilogue):
            """Per dst-block softmax + aggregation.  epilogue(b, h_blk_sbuf_f32)."""
            with tc.tile_pool(name="eFS", bufs=4) as pFS, \
                 tc.tile_pool(name="eFD", bufs=4) as pFD, \
                 tc.tile_pool(name="eU", bufs=3) as pU, \
                 tc.tile_pool(name="eSC", bufs=3) as pSC, \
                 tc.tile_pool(name="eSm", bufs=6) as pSm, \
                 tc.tile_pool(name="eW", bufs=4) as pW, \
                 tc.tile_pool(name="psN", bufs=1, space="PSUM") as psN, \
                 tc.tile_pool(name="psS", bufs=1, space="PSUM") as psS, \
                 tc.tile_pool(name="eZ", bufs=1) as pZ, \
                 tc.tile_pool(name="psT", bufs=2, space="PSUM") as psT:
                for b in range(NBLK):
                    nps = [psN.tile([128, D], dt.float32, name=f"num{h}")
                           for h in range(H)]
                    sps = psS.tile([128, H], dt.float32, name="sps")
                    for t in range(T_BLK):
                        r0 = (b * T_BLK + t) * 128
                        ipk = pSm.tile([128, 4], dt.int32, name="ipk")
                        nc.sync.dma_start(out=ipk[:], in_=idxpk[r0:r0 + 128, :])
                        dl_t = pSm.tile([128, 1], dt.float32, name="dl")
                        nc.vector.tensor_copy(dl_t[:], ipk[:, 2:3])
                        FS = pFS.tile([128, wrow], dt.bfloat16, name="FS")
                        nc.gpsimd.indirect_dma_start(
                            out=FS[:], out_offset=None, in_=FSfull[:],
                            in_offset=bass.IndirectOffsetOnAxis(ap=ipk[:, 0:1], axis=0))
                        FD = pFD.tile([128, wrow], dt.bfloat16, name="FD")
                        nc.gpsimd.indirect_dma_start(
                            out=FD[:], out_offset=None, in_=FDtab[:],
                            in_offset=bass.IndirectOffsetOnAxis(ap=ipk[:, 1:2], axis=0))
                        u = pU.tile([128, wrow], dt.bfloat16, name="u")
                        nc.vector.tensor_tensor(out=u[:], in0=FS[:], in1=FD[:],
                                                op=ALU.add)
                        scr = pSC.tile([128, nfeat], dt.bfloat16, name="scr")
                        acc = pSm.tile([128, 2 * H], dt.float32, name="acc")
                        for h in range(H):
                            sl = [(h * D, h * D + npos[h], h),
                                  (h * D + npos[h], (h + 1) * D, H + h)]
                            for (a0, a1, col) in sl:
                                if a0 == a1:
                                    nc.vector.memset(acc[:, col:col + 1], 0.0)
                                else:
                                    nc.scalar.activation(
                                        scr[:, a0:a1], u[:, a0:a1], AF.Abs,
                                        accum_out=acc[:, col:col + 1])
                        d4 = pSm.tile([128, H], dt.float32, name="d4")
                        nc.vector.tensor_tensor(out=d4[:], in0=acc[:, 0:H],
                                                in1=acc[:, H:2 * H], op=ALU.subtract)
                        t2 = pSm.tile([128, H], dt.float32, name="t2")
                        nc.vector.scalar_tensor_tensor(
                            out=t2[:], in0=u[:, nfeat:nfeat + H], scalar=1.5,
                            in1=d4[:], op0=ALU.mult, op1=ALU.add)
                        wv = pSm.tile([128, H], dt.float32, name="wv")
                        nc.scalar.activation(wv[:], t2[:], AF.Exp, scale=0.4)
                        wvb = pSm.tile([128, H], dt.bfloat16, name="wvb")
                        nc.vector.tensor_copy(wvb[:], wv[:])
                        mask = pW.tile([128, 128], dt.bfloat16, name="mask")
                        nc.vector.tensor_scalar(
                            out=mask[:], in0=iota_f[:],
                            scalar1=dl_t[:, 0:1], scalar2=None,
                            op0=ALU.is_equal)
                        nc.tensor.matmul(sps[:], lhsT=mask[:], rhs=wvb[:],
                                         start=(t == 0), stop=(t == T_BLK - 1))
                        for h in range(H):
                            wsel = pW.tile([128, 128], dt.bfloat16, name=f"wsel{h}")
                            nc.vector.tensor_scalar(
                                out=wsel[:], in0=iota_f[:],
                                scalar1=dl_t[:, 0:1], scalar2=wv[:, h:h + 1],
                                op0=ALU.is_equal, op1=ALU.mult)
                            nc.tensor.matmul(nps[h][:], lhsT=wsel[:],
                                             rhs=FS[:, h * D:(h + 1) * D],
                                             start=(t == 0), stop=(t == T_BLK - 1))
                    # ---- block epilogue: h_blk = relu(num/s * inva + res)
                    seps = pSm.tile([128, H], dt.float32, name="seps")
                    nc.vector.tensor_scalar(out=seps[:], in0=sps[:], scalar1=1e-20,
                                            scalar2=None, op0=ALU.add)
                    rcp = pSm.tile([128, H], dt.float32, name="rcp")
                    nc.vector.reciprocal(rcp[:], seps[:])
                    z = pZ.tile([128, nfeat], dt.float32, name="z")
                    for h in range(H):
                        nc.scalar.activation(z[:, h * D:(h + 1) * D], nps[h][:],
                                             AF.Copy, scale=rcp[:, h:h + 1])
                    zi = pZ.tile([128, nfeat], dt.float32, name="zi")
                    nc.vector.tensor_tensor(out=zi[:], in0=z[:], in1=inva_sb[:],
                                            op=ALU.mult)
                    res_b = pZ.tile([128, nfeat], dt.float32, name="resb")
                    nc.sync.dma_start(out=res_b[:],
                                      in_=RES[b * 128:(b + 1) * 128, :])
                    zr = pZ.tile([128, nfeat], dt.float32, name="zr")
                    nc.vector.tensor_tensor(out=zr[:], in0=zi[:], in1=res_b[:],
                                            op=ALU.add)
                    hb = pZ.tile([128, nfeat], dt.bfloat16, name="hb")
                    nc.scalar.activation(hb[:], zr[:], AF.Relu)
                    epilogue(b, hb, psT, pZ)

        # ============ PHASE C: layer-1 edges ============
        def epi1(b, hb, psT, pZ):
            # transpose [128, F1] -> h1T dram (single consolidated store)
            htall = pZ.tile([128, F1], dt.bfloat16, name="htall")
            for j in range(F1 // 128):
                pt = psT.tile([128, 128], dt.bfloat16, name="pt")
                nc.tensor.transpose(pt[:], hb[:, j * 128:(j + 1) * 128], ident_bf[:])
                nc.scalar.activation(htall[:, j * 128:(j + 1) * 128], pt[:], AF.Copy)
            h1T_view = h1T[:, b * 128:(b + 1) * 128].rearrange(
                "(j r) n -> r j n", r=128)
            nc.sync.dma_start(out=h1T_view, in_=htall[:])

        edge_phase(fs1_full, fd1q, res1, D1, W1ROW, npos1, inva1_sb, F1, epi1)

        # ============ PHASE D: layer-2 projections ============
        with tc.tile_pool(name="psD", bufs=2, space="PSUM") as psD, \
             tc.tile_pool(name="stD", bufs=2) as stDp, \
             tc.tile_pool(name="lhD", bufs=3) as lhDp:
            for nt in range(NBLK):
                pd = psD.tile([128, W2CAT], dt.float32, name="pd")
                lh = lhDp.tile([128, F1], dt.bfloat16, name="lh")
                nc.sync.dma_start(
                    out=lh[:],
                    in_=h1T[:, nt * 128:(nt + 1) * 128].rearrange(
                        "(j r) n -> r j n", r=128))
                for j in range(F1 // 128):
                    nc.tensor.matmul(pd[:], lhsT=lh[:, j * 128:(j + 1) * 128],
                                     rhs=W2_sb[j][:],
                                     start=(j == 0), stop=(j == F1 // 128 - 1))
                stD = stDp.tile([128, 2 * W2ROW], dt.bfloat16, name="stD")
                nc.vector.tensor_tensor(out=stD[:], in0=pd[:, 0:2 * W2ROW],
                                        in1=b2B[:, 0:2 * W2ROW], op=ALU.add)
                stR2 = stDp.tile([128, F2], dt.float32, name="stR2")
                nc.vector.tensor_tensor(out=stR2[:], in0=pd[:, 2 * W2ROW:W2CAT],
                                        in1=b2B[:, 2 * W2ROW:W2CAT], op=ALU.add)
                rows = slice(nt * 128, (nt + 1) * 128)
                nc.sync.dma_start(out=cc2_in[rows, :], in_=stD[:, 0:W2ROW])
                nc.sync.dma_start(out=fd2q[rows, :], in_=stD[:, W2ROW:2 * W2ROW])
                nc.sync.dma_start(out=res2[rows, :], in_=stR2[:])

        # ============ PHASE E: AllGather layer-2 ============
        nc.gpsimd.collective_compute(
            "AllGather", mybir.AluOpType.bypass, replica_groups=groups,
            ins=[cc2_in[:]], outs=[fs2_full[:]])

        # ============ PHASE F: layer-2 edges + output proj ============
        with tc.tile_pool(name="psO", bufs=1, space="PSUM") as psO, \
             tc.tile_pool(name="oSB", bufs=2) as oSB:
            def epi2(b, hb, psT, pZ):
                pt = psT.tile([128, 128], dt.bfloat16, name="pt2")
                nc.tensor.transpose(pt[:], hb[:], ident_bf[:])
                ht = pZ.tile([128, 128], dt.bfloat16, name="ht2")
                nc.scalar.activation(ht[:], pt[:], AF.Copy)
                po = psO.tile([128, OUT], dt.float32, name="po")
                nc.tensor.matmul(po[:], lhsT=ht[:], rhs=Wout_sb[:],
                                 start=True, stop=True)
                ob = oSB.tile([128, OUT], dt.float32, name="ob")
                nc.vector.tensor_tensor(out=ob[:], in0=po[:], in1=boutB[:],
                                        op=ALU.add)
                nc.sync.dma_start(out=out_d[b * 128:(b + 1) * 128, :], in_=ob[:])

            edge_phase(fs2_full, fd2q, res2, D2, W2ROW, npos2, inva2_sb, F2, epi2)

    nc.compile()
    return nc


# -------------------------------------------------------------------- kernel
def _prepare(inputs):
    inputs = {k: np.asarray(v) for k, v in inputs.items()}
    f32 = np.float32

    feat = inputs["features"].astype(f32)
    src = inputs["src"].astype(np.int64)
    dst = inputs["dst"].astype(np.int64)

    (perm1, npos1, W1sp, b1sp, W1dp, b1dp, w1p, b1p, w1q, b1q, inva1) = _fold(
        np.asarray(inputs["attn1"], f32), inputs["W1s"].astype(f32),
        inputs["b1s"].astype(f32), inputs["W1d"].astype(f32),
        inputs["b1d"].astype(f32))
    W1r_p = inputs["W1r"].astype(f32)[:, perm1]
    b1r_p = inputs["b1r"].astype(f32)[perm1]

    (perm2, npos2, W2sp, b2sp, W2dp, b2dp, w2p, b2p, w2q, b2q, inva2) = _fold(
        np.asarray(inputs["attn2"], f32), inputs["W2s"].astype(f32)[perm1],
        inputs["b2s"].astype(f32), inputs["W2d"].astype(f32)[perm1],
        inputs["b2d"].astype(f32))
    W2r_p = inputs["W2r"].astype(f32)[perm1][:, perm2]
    b2r_p = inputs["b2r"].astype(f32)[perm2]
    Wout_p = inputs["Wout"].astype(f32)[perm2]
    bout_v = inputs["bout"].astype(f32)

    W1cat = np.concatenate([W1sp, w1p, W1dp, w1q, W1r_p], 1)          # [128, 6152]
    b1cat = np.concatenate([b1sp, b1p, b1dp, b1q, b1r_p])[None, :]
    W2cat = np.concatenate([W2sp, w2p, W2dp, w2q, W2r_p], 1)          # [2048, 392]
    b2cat = np.concatenate([b2sp, b2p, b2dp, b2q, b2r_p])[None, :]

    t_blk, edge_pc = _edge_arrays(src, dst)

    key = (t_blk, tuple(npos1), tuple(npos2))
    if key not in _cache:
        _cache[key] = _build(dict(t_blk=t_blk, npos1=npos1, npos2=npos2))
    nc = _cache[key]

    in_maps = []
    for c in range(C):
        h0 = np.zeros((NLP, IN), f32)
        h0[:NL] = feat[c * NL:(c + 1) * NL]
        m = dict(
            h0T=np.ascontiguousarray(h0.T),
            W1cat=W1cat, b1cat=b1cat,
            W2cat=W2cat.astype(BF), b2cat=b2cat,
            Wout=Wout_p.astype(BF), bout=bout_v[None, :],
            inva1=inva1[None, :], inva2=inva2[None, :],
            **edge_pc[c],
        )
        in_maps.append(m)
    return nc, in_maps


def kernel(**inputs):
    nc, in_maps = _prepare(inputs)
    from concourse.bass_utils import run_bass_kernel_spmd
    res = run_bass_kernel_spmd(nc, in_maps, core_ids=list(range(C)))
    out = np.concatenate([res.results[c]["out"][:NL] for c in range(C)], 0)
    return out.astype(np.float32)
